# revision 73
# baseline (speedup 1.0000x reference)
"""Trainium2 Bass kernel for nn_Avey_84679575208507.

Reference computation (B=4, N=4096, D=512, E=2048):
  RMSNorm -> Linear(D,E)+relu^2 -> split head/left/right ->
  cosine-sim attention vs learned positional V -> sigmoid gate ->
  Linear(1536,D) + residual.

Sharding: data-parallel over batch x sequence-half; each of 8 cores owns
(batch b = core//2, rows q0 = (core%2)*2048 .. +2048). Tensors are kept in
transposed layout [feature, token] on chip; x and V are pre-transposed (and
token-rotated so the own block is always columns [0, Q)) on the host, g is
folded into W1.

v4: x ships as bf16 and loads in 8 contiguous strips into a resident tile;
every large matmul (enricher linears, similarity S = xr_n^T xr_n, context
ctx = xr_n_at^T (V*S), and the fuser) runs in fp8-e4m3 with DoubleRow perf
mode (2 k-tiles contracted per instruction); V is pre-scaled by 64 into fp8
(undone in the sigmoid's scale). Phases A/B1 (RMSNorm + right path) are
software-pipelined 3-6 blocks deep per 512-token block with on-chip PE
rank-1 broadcasts for the per-token scales; the head/left linears interleave
as PE fill work. Phase C trails the ctx accumulation one key-pair behind the
S matmuls, splits the V*S elementwise product 2:1 across DVE and
(ACT-copy + Pool), and pipelines each query block's sigmoid/gate/fuser tail
inside the next block's key loop; the fuser + residual accumulate in a
single PSUM group per output tile. End-to-end rel err ~1.1e-2 (CPU-model
verified) against the fp32 reference, within the 2e-2 gate.
"""

import sys

sys.path.insert(0, "/opt/trn_rl_repo")

import numpy as np
import ml_dtypes

import concourse.bass as bass
import concourse.tile as tile
import concourse.mybir as mybir
from concourse.bass_utils import run_bass_kernel_spmd

f32 = mybir.dt.float32
bf16 = mybir.dt.bfloat16
fp8 = mybir.dt.float8e4
AF = mybir.ActivationFunctionType
PM = mybir.MatmulPerfMode.DoubleRow
BF = ml_dtypes.bfloat16
F8 = ml_dtypes.float8_e4m3

B, N, D = 4, 4096, 512
E = 4 * D          # 2048
TAIL = E // 2      # 1024
HALF = TAIL // 2   # 512
HEAD = E - TAIL    # 1024
EPS_RMS = 1e-6
Q = N // 2         # 2048 own rows per core
NC = 8
DCH = D // 128     # 4 partition chunks of d
N512 = N // 512    # 8 token blocks
Q512 = Q // 512    # 4 own token blocks
KCH = N // 128     # 32 key chunks
KP = KCH // 2      # 16 key-chunk pairs
EH = HEAD // 128   # 8 head e' chunks
EL = HALF // 128   # 4 left e' chunks
VS = 64.0          # host pre-scale on V (fp8 range); undone in sigmoid scale


def _split_multi_waits(nc):
    """Walrus in this container accepts only one sync-wait per instruction;
    hoist extra waits onto single-wait NoOps just before, same engine."""
    n = 0
    for fn in nc.m.functions:
        for blk in fn.blocks:
            out = []
            for inst in blk.instructions:
                si = inst.sync_info
                if si is not None and si.on_wait and len(si.on_wait) > 1:
                    waits = list(si.on_wait)
                    for i, w in enumerate(waits[:-1]):
                        out.append(mybir.InstNoOp(
                            name=f"{inst.name}_wsplit{i}",
                            engine=inst.engine,
                            bass_nofuse=True,
                            sync_info=mybir.SyncInfo(on_wait=[w], on_update=[]),
                        ))
                    inst.sync_info = mybir.SyncInfo(
                        on_wait=[waits[-1]], on_update=list(si.on_update or []))
                    n += 1
                out.append(inst)
            blk.instructions = out
    return n


def _build(phases=2):
    nc = _build_inner(phases)
    _split_multi_waits(nc)
    return nc


def _build_inner(phases=2):
    nc = bass.Bass("TRN2", target_bir_lowering=False, debug=False, num_devices=NC)

    xT = nc.dram_tensor("xT", [D, N], bf16, kind="ExternalInput").ap()
    # [p, qt, kp, t2, qq] pre-arranged so each C-phase load is contiguous
    vt8 = nc.dram_tensor("vt8", [128, Q512, KP, 2, 512], fp8,
                         kind="ExternalInput").ap()
    # weights pre-arranged host-side to [128, chunks*cols] contiguous rows
    w1h = nc.dram_tensor("w1h", [128, DCH * HEAD], fp8, kind="ExternalInput").ap()
    w1l = nc.dram_tensor("w1l", [128, DCH * HALF], fp8, kind="ExternalInput").ap()
    w1r = nc.dram_tensor("w1r", [128, DCH * HALF], fp8, kind="ExternalInput").ap()
    wfh = nc.dram_tensor("wfh", [128, EH * D], fp8, kind="ExternalInput").ap()
    wfg = nc.dram_tensor("wfg", [128, EL * D], fp8, kind="ExternalInput").ap()
    b1h = nc.dram_tensor("b1h", [128, EH], f32, kind="ExternalInput").ap()
    b1l = nc.dram_tensor("b1l", [128, EL], f32, kind="ExternalInput").ap()
    b1r = nc.dram_tensor("b1r", [128, EL], f32, kind="ExternalInput").ap()
    biasq = nc.dram_tensor("biasq", [128, DCH], f32, kind="ExternalInput").ap()
    id8 = nc.dram_tensor("id8", [128, 128], fp8, kind="ExternalInput").ap()
    idb = nc.dram_tensor("idb", [128, 128], bf16, kind="ExternalInput").ap()
    onesb = nc.dram_tensor("onesb", [128, 1], bf16, kind="ExternalInput").ap()
    onesr = nc.dram_tensor("onesr", [1, 128], bf16, kind="ExternalInput").ap()
    epsb = nc.dram_tensor("epsb", [128, 2], f32, kind="ExternalInput").ap()
    outT = nc.dram_tensor("outT", [D, Q], f32, kind="ExternalOutput").ap()

    with tile.TileContext(nc) as tc:
        with (
            tc.tile_pool(name="consts", bufs=1) as consts,
            tc.tile_pool(name="wfp", bufs=1) as wfp,
            tc.tile_pool(name="xr8Tp", bufs=1) as xr8Tp,
            tc.tile_pool(name="xr8atp", bufs=1) as xr8atp,
            tc.tile_pool(name="xlTp", bufs=1) as xlTp,
            tc.tile_pool(name="headTp", bufs=1) as headTp,
            tc.tile_pool(name="xofp", bufs=1) as xofp,
        ):
            id8_t = consts.tile([128, 128], fp8)
            nc.sync.dma_start(id8_t[:], id8[:])
            idb_t = consts.tile([128, 128], bf16)
            nc.sync.dma_start(idb_t[:], idb[:])
            ones_t = consts.tile([128, 1], bf16)
            nc.sync.dma_start(ones_t[:], onesb[:])
            onesr_t = consts.tile([1, 128], bf16)
            nc.sync.dma_start(onesr_t[:], onesr[:])
            b1h_t = consts.tile([128, EH], f32)
            nc.sync.dma_start(b1h_t[:], b1h[:])
            b1l_t = consts.tile([128, EL], f32)
            nc.sync.dma_start(b1l_t[:], b1l[:])
            b1r_t = consts.tile([128, EL], f32)
            nc.sync.dma_start(b1r_t[:], b1r[:])
            bq_t = consts.tile([128, DCH], f32)
            nc.sync.dma_start(bq_t[:], biasq[:])
            eps_t = consts.tile([128, 2], f32)
            nc.sync.dma_start(eps_t[:], epsb[:])
            wfh_t = wfp.tile([128, EH, D], fp8)
            wfg_t = wfp.tile([128, EL, D], fp8)

            xr8T = xr8Tp.tile([128, DCH, N], fp8)
            xr8at = xr8atp.tile([128, KCH, HALF], fp8)
            xlT = xlTp.tile([128, EL, Q], bf16)
            headT = headTp.tile([128, EH, Q], fp8)
            xall = xofp.tile([128, DCH, N], bf16)
            for i_ in range(DCH):
                for h_ in range(2):
                    nc.scalar.dma_start(
                        xall[:, i_, 2048 * h_:2048 * (h_ + 1)],
                        xT[128 * i_:128 * (i_ + 1),
                           2048 * h_:2048 * (h_ + 1)])

            # ============ Phase AB: pipelined RMSNorm + right path +
            # ============ head/left, per 512-token block j ============
            with (
                tc.tile_pool(name="w1p", bufs=1) as w1p,
                tc.tile_pool(name="xnp", bufs=1) as xnp,
                tc.tile_pool(name="sqp", bufs=3) as sqp,
                tc.tile_pool(name="xrp", bufs=5) as xrp,
                tc.tile_pool(name="trp", bufs=3) as trp,
                tc.tile_pool(name="trcp", bufs=2) as trcp,
                tc.tile_pool(name="rowsb", bufs=2) as rowsb,
                tc.tile_pool(name="rowps", bufs=2, space="PSUM") as rowps,
                tc.tile_pool(name="psb", bufs=4, space="PSUM") as psb,
                tc.tile_pool(name="trps", bufs=2, space="PSUM") as trps,
            ):
                w1h_t = w1p.tile([128, DCH, HEAD], fp8)
                w1l_t = w1p.tile([128, DCH, HALF], fp8)
                w1r_t = w1p.tile([128, DCH, HALF], fp8)
                xnT = xnp.tile([128, DCH, N], fp8)

                xc_t = {}
                xr_t = {}
                ms_t = {}
                rsl_t = {}

                def A1(j):
                    jsl = slice(512 * j, 512 * (j + 1))
                    ss = rowps.tile([1, 512], f32, tag="row")
                    xc = xall[:, :, jsl]
                    xc_t[j] = xc
                    xsq = sqp.tile([128, DCH, 512], bf16, tag="xsq")
                    for i in range(DCH):
                        nc.vector.tensor_mul(xsq[:, i], xc[:, i], xc[:, i])
                        nc.tensor.matmul(ss[0:1, :], ones_t[:], xsq[:, i],
                                         start=(i == 0), stop=(i == DCH - 1))
                    ms = rowsb.tile([1, 512], f32, tag="rowf")
                    nc.scalar.activation(ms[:], ss[0:1, :], AF.Identity,
                                         bias=eps_t[0:1, 0:1], scale=1.0 / D)
                    ms_t[j] = ms

                def A2(j):
                    jsl = slice(512 * j, 512 * (j + 1))
                    rr = rowsb.tile([1, 512], f32, tag="rowf2")
                    nc.vector.reciprocal(rr[:], ms_t.pop(j)[:])
                    sr = rowsb.tile([1, 512], bf16, tag="rowb")
                    nc.scalar.sqrt(sr[:], rr[:])
                    # broadcast row across partitions on PE: ones[1,128] x row
                    sb = psb.tile([128, 512], f32, tag="ps")
                    nc.tensor.matmul(sb[:], onesr_t[:], sr[:],
                                     start=True, stop=True)
                    xc = xc_t.pop(j)
                    nc.vector.tensor_mul(
                        xnT[:, :, jsl], xc,
                        sb[:].rearrange("p (one q) -> p one q",
                                        one=1).broadcast_to([128, DCH, 512]))

                def B1a(j):
                    jsl = slice(512 * j, 512 * (j + 1))
                    rs = rowps.tile([1, 512], f32, tag="row")
                    tr = trp.tile([128, DCH, 512], bf16, tag="tr")
                    for dR in range(DCH):
                        ps = psb.tile([128, 512], f32, tag="ps")
                        for t in range(2):
                            nc.tensor.matmul(
                                ps[:],
                                w1r_t[:, 2 * t:2 * t + 2,
                                      128 * dR:128 * (dR + 1)],
                                xnT[:, 2 * t:2 * t + 2, jsl],
                                start=(t == 0), stop=(t == 1), perf_mode=PM)
                        nc.scalar.activation(tr[:, dR], ps[:], AF.Relu,
                                             bias=b1r_t[:, dR:dR + 1])
                    xr = xrp.tile([128, DCH, 512], bf16, tag="xr")
                    xr_t[j] = xr
                    xq = sqp.tile([128, DCH, 512], bf16, tag="xsq")
                    for dR in range(DCH):
                        nc.vector.tensor_mul(xr[:, dR], tr[:, dR], tr[:, dR])
                        nc.gpsimd.tensor_mul(xq[:, dR], xr[:, dR], xr[:, dR])
                    for dR in range(DCH):
                        nc.tensor.matmul(rs[0:1, :], ones_t[:], xq[:, dR],
                                         start=(dR == 0), stop=(dR == DCH - 1))
                    rsl = rowsb.tile([1, 512], f32, tag="rowf")
                    # 1/max(sqrt(S),1e-12) == 1/sqrt(S+1e-24) in fp32
                    nc.scalar.activation(rsl[:], rs[0:1, :], AF.Identity,
                                         bias=eps_t[0:1, 1:2])
                    rsl_t[j] = rsl

                def B1b(j):
                    jsl = slice(512 * j, 512 * (j + 1))
                    rc = rowsb.tile([1, 512], f32, tag="rowf2")
                    nc.vector.reciprocal(rc[:], rsl_t.pop(j)[:])
                    rq = rowsb.tile([1, 512], bf16, tag="rowb")
                    nc.scalar.sqrt(rq[:], rc[:])
                    rb = psb.tile([128, 512], f32, tag="ps")
                    nc.tensor.matmul(rb[:], onesr_t[:], rq[:],
                                     start=True, stop=True)
                    xr = xr_t.pop(j)
                    nc.vector.tensor_mul(
                        xr8T[:, :, jsl], xr[:],
                        rb[:].rearrange("p (one q) -> p one q",
                                        one=1).broadcast_to([128, DCH, 512]))

                def TR(j):
                    for kk in range(4):
                        k = 4 * j + kk
                        tp = trps.tile([128, DCH, 256], fp8, tag="trp")
                        for dR in range(DCH):
                            outap = tp[:, dR].rearrange(
                                "p (c two) -> p c two", two=2)[:, :, 0]
                            nc.tensor.transpose(
                                outap, xr8T[:, dR, 128 * k:128 * (k + 1)],
                                id8_t[:])
                        src = tp[:].rearrange(
                            "p c (m two) -> p c m two", two=2)[:, :, :, 0]
                        dst = xr8at[:, k, :].rearrange(
                            "p (c m) -> p c m", c=DCH)
                        nc.scalar.activation(dst, src, AF.Copy)

                def B2(c):
                    # chunk c of 8: jq = c//2; half 0 = eh 0-5,
                    # half 1 = eh 6-7 + el 0-3
                    jq, hf = divmod(c, 2)
                    qsl = slice(512 * jq, 512 * (jq + 1))
                    groups = ([("h", e) for e in range(6)] if hf == 0 else
                              [("h", 6), ("h", 7)] + [("l", e)
                                                      for e in range(EL)])
                    trc = trcp.tile([128, 6, 512], bf16, tag="trc")
                    for gi, (kind, e) in enumerate(groups):
                        wt_, bt_ = (w1h_t, b1h_t) if kind == "h" else \
                                   (w1l_t, b1l_t)
                        ps = psb.tile([128, 512], f32, tag="ps")
                        for t in range(2):
                            nc.tensor.matmul(
                                ps[:],
                                wt_[:, 2 * t:2 * t + 2,
                                    128 * e:128 * (e + 1)],
                                xnT[:, 2 * t:2 * t + 2, qsl],
                                start=(t == 0), stop=(t == 1), perf_mode=PM)
                        nc.scalar.activation(trc[:, gi], ps[:], AF.Relu,
                                             bias=bt_[:, e:e + 1])
                    for gi, (kind, e) in enumerate(groups):
                        if kind == "h":
                            eng = nc.gpsimd if e % 3 != 2 else nc.vector
                            eng.tensor_mul(headT[:, e, qsl],
                                           trc[:, gi], trc[:, gi])
                        else:
                            nc.vector.tensor_mul(xlT[:, e, qsl],
                                                 trc[:, gi], trc[:, gi])

                # x blocks 0-1 queued before the bulk weight loads so compute
                # can start immediately; fuser weights load at the end of AB.
                A1(0)
                nc.sync.dma_start(w1r_t[:],
                                  w1r.rearrange("p (c m) -> p c m", c=DCH))
                A1(1)
                A1(2)
                nc.sync.dma_start(w1h_t[:],
                                  w1h.rearrange("p (c m) -> p c m", c=DCH))
                nc.sync.dma_start(w1l_t[:],
                                  w1l.rearrange("p (c m) -> p c m", c=DCH))
                for u in range(3, 12):
                    if u <= 10:
                        A2(u - 3)
                        B1a(u - 3)
                    if 6 <= u:
                        B1b(u - 6)
                        TR(u - 6)
                    if u <= 7:
                        A1(u)
                    if 4 <= u <= 11:
                        B2(u - 4)
                    if u == 7:
                        nc.sync.dma_start(
                            wfh_t[:], wfh.rearrange("p (c m) -> p c m", c=EH))
                        nc.sync.dma_start(
                            wfg_t[:], wfg.rearrange("p (c m) -> p c m", c=EL))
                B1b(6)
                TR(6)
                B1b(7)
                TR(7)

            # ============ Phase C: similarity, context, gate, fuser ========
            if phases < 2:
                return nc
            with (
                tc.tile_pool(name="ctxps", bufs=1, space="PSUM") as ctxps,
                tc.tile_pool(name="stp", bufs=4, space="PSUM") as stp,
                tc.tile_pool(name="vtp", bufs=9) as vtp,
                tc.tile_pool(name="wtp", bufs=8) as wtp,
                tc.tile_pool(name="stsb", bufs=6) as stsb,
                tc.tile_pool(name="csp", bufs=3) as csp,
                tc.tile_pool(name="gtp", bufs=3) as gtp,
                tc.tile_pool(name="outp", bufs=8) as outp,
            ):
                vt_q = {}
                ctxs = {}
                gts = {}

                def tail(qtp):
                    # sigmoid + gate for a finished qt block
                    qslp = slice(512 * qtp, 512 * (qtp + 1))
                    ctxp = ctxs[qtp]
                    cs = csp.tile([128, DCH, 512], bf16, tag="cs")
                    for dO in range(DCH):
                        nc.scalar.activation(cs[:, dO], ctxp[:, dO],
                                             AF.Sigmoid,
                                             bias=bq_t[:, dO:dO + 1],
                                             scale=1.0 / VS)
                    gt = gtp.tile([128, EL, 512], fp8, tag="gt")
                    for dO in range(DCH):
                        nc.vector.tensor_mul(gt[:, dO], xlT[:, dO, qslp],
                                             cs[:, dO])
                    gts[qtp] = gt

                def fuser_out(qtp):
                    qslp = slice(512 * qtp, 512 * (qtp + 1))
                    gt = gts.pop(qtp)
                    ctxs.pop(qtp)
                    for do in range(DCH):
                        fgd = stp.tile([128, 512], f32, tag="st")
                        for e2 in range(EH // 2):
                            nc.tensor.matmul(
                                fgd[:],
                                wfh_t[:, 2 * e2:2 * e2 + 2,
                                      128 * do:128 * (do + 1)],
                                headT[:, 2 * e2:2 * e2 + 2, qslp],
                                start=(e2 == 0), stop=False, perf_mode=PM)
                        for l2 in range(EL // 2):
                            nc.tensor.matmul(
                                fgd[:],
                                wfg_t[:, 2 * l2:2 * l2 + 2,
                                      128 * do:128 * (do + 1)],
                                gt[:, 2 * l2:2 * l2 + 2, :],
                                start=False, stop=False, perf_mode=PM)
                        nc.tensor.matmul(
                            fgd[:], idb_t[:], xall[:, do, qslp],
                            start=False, stop=True)
                        ob = outp.tile([128, 512], f32, tag="ob")
                        nc.scalar.activation(ob[:], fgd[:], AF.Copy)
                        nc.sync.dma_start(
                            outT[128 * do:128 * (do + 1), qslp], ob[:])

                for qt in range(Q512):
                    qsl = slice(512 * qt, 512 * (qt + 1))
                    ctx = ctxps.tile([128, DCH, 512], f32, tag="ctx")
                    ctxs[qt] = ctx
                    for kp in range(4):
                        vt_t = vtp.tile([128, 2, 512], fp8, tag="vt")
                        nc.scalar.dma_start(vt_t[:], vt8[:, qt, kp])
                        vt_q[kp] = vt_t

                    def emit_ctx(kp, wt, ctx=ctx):
                        for dO in range(DCH):
                            nc.tensor.matmul(
                                ctx[:, dO],
                                xr8at[:, 2 * kp:2 * kp + 2,
                                      128 * dO:128 * (dO + 1)],
                                wt[:],
                                start=(kp == 0), stop=(kp == KP - 1),
                                perf_mode=PM)

                    pendq = []
                    for kp in range(KP):
                        if kp + 4 < KP:
                            vt_t = vtp.tile([128, 2, 512], fp8, tag="vt")
                            nc.scalar.dma_start(vt_t[:], vt8[:, qt, kp + 4])
                            vt_q[kp + 4] = vt_t
                        wt = wtp.tile([128, 2, 512], fp8, tag="wt")
                        vt_t = vt_q.pop(kp)
                        for t2 in range(2):
                            k = 2 * kp + t2
                            st = stp.tile([128, 512], f32, tag="st")
                            for t in range(2):
                                nc.tensor.matmul(
                                    st[:],
                                    xr8T[:, 2 * t:2 * t + 2,
                                         128 * k:128 * (k + 1)],
                                    xr8T[:, 2 * t:2 * t + 2, qsl],
                                    start=(t == 0), stop=(t == 1),
                                    perf_mode=PM)
                            if k % 3 == 2:
                                sb2 = stsb.tile([128, 512], bf16, tag="stb")
                                nc.scalar.activation(sb2[:], st[:], AF.Copy)
                                nc.gpsimd.tensor_mul(wt[:, t2], sb2[:],
                                                     vt_t[:, t2])
                            else:
                                nc.vector.tensor_mul(wt[:, t2], st[:],
                                                     vt_t[:, t2])
                        pendq.append((kp, wt))
                        if len(pendq) > 5:
                            emit_ctx(*pendq.pop(0))
                        if qt > 0 and kp == 1:
                            tail(qt - 1)
                        if qt > 0 and kp == 4:
                            fuser_out(qt - 1)
                    while pendq:
                        emit_ctx(*pendq.pop(0))
                tail(Q512 - 1)
                fuser_out(Q512 - 1)

    return nc


_NC_CACHE = {}


def _get_nc(phases=2):
    if phases not in _NC_CACHE:
        _NC_CACHE[phases] = _build(phases)
    return _NC_CACHE[phases]


def _chunk_rows(w, nch):
    """[D=nch*128, M] -> [128, nch*M] so row p holds chunks c-major."""
    ncols = w.shape[1]
    return np.ascontiguousarray(
        w.reshape(nch, 128, ncols).transpose(1, 0, 2).reshape(128, nch * ncols))


def _prep_inputs(x, g, W1, b1, V, bias, Wf):
    x = np.asarray(x, dtype=np.float32)
    g = np.asarray(g, dtype=np.float32)
    W1 = np.asarray(W1, dtype=np.float32)
    b1 = np.asarray(b1, dtype=np.float32)
    V = np.asarray(V, dtype=np.float32)
    bias = np.asarray(bias, dtype=np.float32)
    Wf = np.asarray(Wf, dtype=np.float32)

    W1g = W1 * g[:, None]
    w1h = _chunk_rows(W1g[:, :HEAD].astype(F8), DCH)
    w1l = _chunk_rows(W1g[:, HEAD:HEAD + HALF].astype(F8), DCH)
    w1r = _chunk_rows(W1g[:, HEAD + HALF:].astype(F8), DCH)
    wfh = _chunk_rows(Wf[:HEAD].astype(F8), EH)
    wfg = _chunk_rows(Wf[HEAD:].astype(F8), EL)
    b1h = np.ascontiguousarray(b1[:HEAD].reshape(EH, 128).T)
    b1l = np.ascontiguousarray(b1[HEAD:HEAD + HALF].reshape(EL, 128).T)
    b1r = np.ascontiguousarray(b1[HEAD + HALF:].reshape(EL, 128).T)
    biasq = np.ascontiguousarray(bias.reshape(DCH, 128).T)
    id8_np = np.eye(128, dtype=F8)
    idb_np = np.eye(128, dtype=BF)
    ones_np = np.ones((128, 1), dtype=BF)
    onesr_np = np.ones((1, 128), dtype=BF)
    epsb_np = np.tile(np.array([[EPS_RMS, 1e-24]], np.float32), (128, 1))
    VT = np.ascontiguousarray(VS * V.T).astype(F8)   # VT[k, q] = 64*V[q, k]

    in_maps = []
    for c in range(NC):
        b, h = divmod(c, 2)
        q0 = h * Q
        xTb = x[b].T  # [D, N]
        if q0 == 0:
            xrot = np.ascontiguousarray(xTb).astype(BF)
            vrot = VT[:, :Q]
        else:
            # rotate tokens so own block is first; V rows rotate identically
            xrot = np.ascontiguousarray(
                np.concatenate([xTb[:, q0:], xTb[:, :q0]], axis=1)).astype(BF)
            vrot = np.concatenate([VT[q0:, q0:], VT[:q0, q0:]], axis=0)
        # [k, q] -> [p, qt, kp, t2, qq]; k = kp*256 + t2*128 + p, q = qt*512+qq
        v5 = np.ascontiguousarray(
            vrot.reshape(KP, 2, 128, Q512, 512).transpose(2, 3, 0, 1, 4))
        in_maps.append({
            "xT": xrot, "vt8": v5,
            "w1h": w1h, "w1l": w1l, "w1r": w1r,
            "wfh": wfh, "wfg": wfg,
            "b1h": b1h, "b1l": b1l, "b1r": b1r,
            "biasq": biasq, "id8": id8_np, "idb": idb_np, "onesb": ones_np,
            "onesr": onesr_np, "epsb": epsb_np,
        })
    return in_maps


def _run(in_maps, trace=False):
    nc = _get_nc()
    return run_bass_kernel_spmd(nc, in_maps, list(range(NC)), trace=trace)


def _assemble(results):
    out = np.empty((B, N, D), dtype=np.float32)
    for c in range(NC):
        b, h = divmod(c, 2)
        q0 = h * Q
        out[b, q0:q0 + Q, :] = results[c]["outT"].T
    return out


def kernel(x, g, W1, b1, V, bias, Wf):
    in_maps = _prep_inputs(x, g, W1, b1, V, bias, Wf)
    res = _run(in_maps, trace=False)
    return _assemble(res.results)


def kernel_traced(x, g, W1, b1, V, bias, Wf):
    """Same as kernel() but with NTFF tracing; returns (out, results)."""
    in_maps = _prep_inputs(x, g, W1, b1, V, bias, Wf)
    res = _run(in_maps, trace=True)
    return _assemble(res.results), res


# revision 77
# speedup vs baseline: 1.3874x; 1.3874x over previous
"""Trainium2 Bass kernel for nn_Avey_84679575208507.

Reference computation (B=4, N=4096, D=512, E=2048):
  RMSNorm -> Linear(D,E)+relu^2 -> split head/left/right ->
  cosine-sim attention vs learned positional V -> sigmoid gate ->
  Linear(1536,D) + residual.

Sharding: data-parallel over batch x sequence-half; each of 8 cores owns
(batch b = core//2, rows q0 = (core%2)*2048 .. +2048). Tensors are kept in
transposed layout [feature, token] on chip; x and V are pre-transposed (and
token-rotated so the own block is always columns [0, Q)) on the host, g is
folded into W1.

v4: x ships as bf16 and loads in 8 contiguous strips into a resident tile;
every large matmul (enricher linears, similarity S = xr_n^T xr_n, context
ctx = xr_n_at^T (V*S), and the fuser) runs in fp8-e4m3 with DoubleRow perf
mode (2 k-tiles contracted per instruction); V is pre-scaled by 64 into fp8
(undone in the sigmoid's scale). Phases A/B1 (RMSNorm + right path) are
software-pipelined 3-6 blocks deep per 512-token block with on-chip PE
rank-1 broadcasts for the per-token scales; the head/left linears interleave
as PE fill work. Phase C trails the ctx accumulation one key-pair behind the
S matmuls, splits the V*S elementwise product 2:1 across DVE and
(ACT-copy + Pool), and pipelines each query block's sigmoid/gate/fuser tail
inside the next block's key loop; the fuser + residual accumulate in a
single PSUM group per output tile. End-to-end rel err ~1.1e-2 (CPU-model
verified) against the fp32 reference, within the 2e-2 gate.
"""

import sys

sys.path.insert(0, "/opt/trn_rl_repo")

import numpy as np
import ml_dtypes

import concourse.bass as bass
import concourse.tile as tile
import concourse.mybir as mybir
from concourse.bass_utils import run_bass_kernel_spmd

f32 = mybir.dt.float32
bf16 = mybir.dt.bfloat16
fp8 = mybir.dt.float8e4
AF = mybir.ActivationFunctionType
PM = mybir.MatmulPerfMode.DoubleRow
BF = ml_dtypes.bfloat16
F8 = ml_dtypes.float8_e4m3

B, N, D = 4, 4096, 512
E = 4 * D          # 2048
TAIL = E // 2      # 1024
HALF = TAIL // 2   # 512
HEAD = E - TAIL    # 1024
EPS_RMS = 1e-6
Q = N // 2         # 2048 own rows per core
NC = 8
DCH = D // 128     # 4 partition chunks of d
N512 = N // 512    # 8 token blocks
Q512 = Q // 512    # 4 own token blocks
KCH = N // 128     # 32 key chunks
KP = KCH // 2      # 16 key-chunk pairs
EH = HEAD // 128   # 8 head e' chunks
EL = HALF // 128   # 4 left e' chunks
VS = 64.0          # host pre-scale on V (fp8 range); undone in sigmoid scale


def _split_multi_waits(nc):
    """Walrus in this container accepts only one sync-wait per instruction;
    hoist extra waits onto single-wait NoOps just before, same engine."""
    n = 0
    for fn in nc.m.functions:
        for blk in fn.blocks:
            out = []
            for inst in blk.instructions:
                si = inst.sync_info
                if si is not None and si.on_wait and len(si.on_wait) > 1:
                    waits = list(si.on_wait)
                    for i, w in enumerate(waits[:-1]):
                        out.append(mybir.InstNoOp(
                            name=f"{inst.name}_wsplit{i}",
                            engine=inst.engine,
                            bass_nofuse=True,
                            sync_info=mybir.SyncInfo(on_wait=[w], on_update=[]),
                        ))
                    inst.sync_info = mybir.SyncInfo(
                        on_wait=[waits[-1]], on_update=list(si.on_update or []))
                    n += 1
                out.append(inst)
            blk.instructions = out
    return n


def _build(phases=2):
    nc = _build_inner(phases)
    _split_multi_waits(nc)
    return nc


def _build_inner(phases=2):
    nc = bass.Bass("TRN2", target_bir_lowering=False, debug=False, num_devices=NC)

    xT = nc.dram_tensor("xT", [D, N], bf16, kind="ExternalInput").ap()
    # [p, qt, kp, t2, qq] pre-arranged so each C-phase load is contiguous
    vt8 = nc.dram_tensor("vt8", [128, Q512, KP, 2, 512], fp8,
                         kind="ExternalInput").ap()
    # weights pre-arranged host-side to [128, chunks*cols] contiguous rows
    w1h = nc.dram_tensor("w1h", [128, DCH * HEAD], fp8, kind="ExternalInput").ap()
    w1l = nc.dram_tensor("w1l", [128, DCH * HALF], fp8, kind="ExternalInput").ap()
    w1r = nc.dram_tensor("w1r", [128, DCH * HALF], fp8, kind="ExternalInput").ap()
    wfh = nc.dram_tensor("wfh", [128, EH * D], fp8, kind="ExternalInput").ap()
    wfg = nc.dram_tensor("wfg", [128, EL * D], fp8, kind="ExternalInput").ap()
    b1h = nc.dram_tensor("b1h", [128, EH], f32, kind="ExternalInput").ap()
    b1l = nc.dram_tensor("b1l", [128, EL], f32, kind="ExternalInput").ap()
    b1r = nc.dram_tensor("b1r", [128, EL], f32, kind="ExternalInput").ap()
    biasq = nc.dram_tensor("biasq", [128, DCH], f32, kind="ExternalInput").ap()
    id8 = nc.dram_tensor("id8", [128, 128], fp8, kind="ExternalInput").ap()
    idb = nc.dram_tensor("idb", [128, 128], bf16, kind="ExternalInput").ap()
    onesb = nc.dram_tensor("onesb", [128, 1], bf16, kind="ExternalInput").ap()
    onesr = nc.dram_tensor("onesr", [1, 128], bf16, kind="ExternalInput").ap()
    epsb = nc.dram_tensor("epsb", [128, 2], f32, kind="ExternalInput").ap()
    outT = nc.dram_tensor("outT", [D, Q], f32, kind="ExternalOutput").ap()

    with tile.TileContext(nc) as tc:
        with (
            tc.tile_pool(name="consts", bufs=1) as consts,
            tc.tile_pool(name="wfp", bufs=1) as wfp,
            tc.tile_pool(name="xr8Tp", bufs=1) as xr8Tp,
            tc.tile_pool(name="xr8atp", bufs=1) as xr8atp,
            tc.tile_pool(name="xlTp", bufs=1) as xlTp,
            tc.tile_pool(name="headTp", bufs=1) as headTp,
            tc.tile_pool(name="xofp", bufs=1) as xofp,
        ):
            id8_t = consts.tile([128, 128], fp8)
            nc.sync.dma_start(id8_t[:], id8[:])
            idb_t = consts.tile([128, 128], bf16)
            nc.sync.dma_start(idb_t[:], idb[:])
            ones_t = consts.tile([128, 1], bf16)
            nc.sync.dma_start(ones_t[:], onesb[:])
            onesr_t = consts.tile([1, 128], bf16)
            nc.sync.dma_start(onesr_t[:], onesr[:])
            b1h_t = consts.tile([128, EH], f32)
            nc.sync.dma_start(b1h_t[:], b1h[:])
            b1l_t = consts.tile([128, EL], f32)
            nc.sync.dma_start(b1l_t[:], b1l[:])
            b1r_t = consts.tile([128, EL], f32)
            nc.sync.dma_start(b1r_t[:], b1r[:])
            bq_t = consts.tile([128, DCH], f32)
            nc.sync.dma_start(bq_t[:], biasq[:])
            eps_t = consts.tile([128, 2], f32)
            nc.sync.dma_start(eps_t[:], epsb[:])
            wfh_t = wfp.tile([128, EH, D], fp8)
            wfg_t = wfp.tile([128, EL, D], fp8)

            xr8T = xr8Tp.tile([128, DCH, N], fp8)
            xr8at = xr8atp.tile([128, KCH, HALF], fp8)
            xlT = xlTp.tile([128, EL, Q], bf16)
            headT = headTp.tile([128, EH, Q], fp8)
            xall = xofp.tile([128, DCH, N], bf16)
            for i_ in range(DCH):
                for h_ in range(2):
                    nc.scalar.dma_start(
                        xall[:, i_, 2048 * h_:2048 * (h_ + 1)],
                        xT[128 * i_:128 * (i_ + 1),
                           2048 * h_:2048 * (h_ + 1)])

            # ============ Phase AB: pipelined RMSNorm + right path +
            # ============ head/left, per 512-token block j ============
            with (
                tc.tile_pool(name="w1p", bufs=1) as w1p,
                tc.tile_pool(name="xnp", bufs=1) as xnp,
                tc.tile_pool(name="sqp", bufs=3) as sqp,
                tc.tile_pool(name="xrp", bufs=5) as xrp,
                tc.tile_pool(name="trp", bufs=3) as trp,
                tc.tile_pool(name="trcp", bufs=2) as trcp,
                tc.tile_pool(name="rowsb", bufs=2) as rowsb,
                tc.tile_pool(name="rowps", bufs=2, space="PSUM") as rowps,
                tc.tile_pool(name="psb", bufs=4, space="PSUM") as psb,
                tc.tile_pool(name="trps", bufs=2, space="PSUM") as trps,
            ):
                w1h_t = w1p.tile([128, DCH, HEAD], fp8)
                w1l_t = w1p.tile([128, DCH, HALF], fp8)
                w1r_t = w1p.tile([128, DCH, HALF], fp8)
                xnT = xnp.tile([128, DCH, N], fp8)

                xc_t = {}
                xr_t = {}
                ms_t = {}
                rsl_t = {}

                def A1(j):
                    jsl = slice(512 * j, 512 * (j + 1))
                    ss = rowps.tile([1, 512], f32, tag="row")
                    xc = xall[:, :, jsl]
                    xc_t[j] = xc
                    xsq = sqp.tile([128, DCH, 512], bf16, tag="xsq")
                    for i in range(DCH):
                        nc.vector.tensor_mul(xsq[:, i], xc[:, i], xc[:, i])
                        nc.tensor.matmul(ss[0:1, :], ones_t[:], xsq[:, i],
                                         start=(i == 0), stop=(i == DCH - 1))
                    ms = rowsb.tile([1, 512], f32, tag="rowf")
                    nc.scalar.activation(ms[:], ss[0:1, :], AF.Identity,
                                         bias=eps_t[0:1, 0:1], scale=1.0 / D)
                    ms_t[j] = ms

                def A2(j):
                    jsl = slice(512 * j, 512 * (j + 1))
                    rr = rowsb.tile([1, 512], f32, tag="rowf2")
                    nc.vector.reciprocal(rr[:], ms_t.pop(j)[:])
                    sr = rowsb.tile([1, 512], bf16, tag="rowb")
                    nc.scalar.sqrt(sr[:], rr[:])
                    # broadcast row across partitions on PE: ones[1,128] x row
                    sb = psb.tile([128, 512], f32, tag="ps")
                    nc.tensor.matmul(sb[:], onesr_t[:], sr[:],
                                     start=True, stop=True)
                    xc = xc_t.pop(j)
                    nc.vector.tensor_mul(
                        xnT[:, :, jsl], xc,
                        sb[:].rearrange("p (one q) -> p one q",
                                        one=1).broadcast_to([128, DCH, 512]))

                def B1a(j):
                    jsl = slice(512 * j, 512 * (j + 1))
                    rs = rowps.tile([1, 512], f32, tag="row")
                    tr = trp.tile([128, DCH, 512], bf16, tag="tr")
                    for dR in range(DCH):
                        ps = psb.tile([128, 512], f32, tag="ps")
                        for t in range(2):
                            nc.tensor.matmul(
                                ps[:],
                                w1r_t[:, 2 * t:2 * t + 2,
                                      128 * dR:128 * (dR + 1)],
                                xnT[:, 2 * t:2 * t + 2, jsl],
                                start=(t == 0), stop=(t == 1), perf_mode=PM)
                        nc.scalar.activation(tr[:, dR], ps[:], AF.Relu,
                                             bias=b1r_t[:, dR:dR + 1])
                    xr = xrp.tile([128, DCH, 512], bf16, tag="xr")
                    xr_t[j] = xr
                    xq = sqp.tile([128, DCH, 512], bf16, tag="xsq")
                    for dR in range(DCH):
                        nc.vector.tensor_mul(xr[:, dR], tr[:, dR], tr[:, dR])
                        nc.gpsimd.tensor_mul(xq[:, dR], xr[:, dR], xr[:, dR])
                    for dR in range(DCH):
                        nc.tensor.matmul(rs[0:1, :], ones_t[:], xq[:, dR],
                                         start=(dR == 0), stop=(dR == DCH - 1))
                    rsl = rowsb.tile([1, 512], f32, tag="rowf")
                    # 1/max(sqrt(S),1e-12) == 1/sqrt(S+1e-24) in fp32
                    nc.scalar.activation(rsl[:], rs[0:1, :], AF.Identity,
                                         bias=eps_t[0:1, 1:2])
                    rsl_t[j] = rsl

                def B1b(j):
                    jsl = slice(512 * j, 512 * (j + 1))
                    rc = rowsb.tile([1, 512], f32, tag="rowf2")
                    nc.vector.reciprocal(rc[:], rsl_t.pop(j)[:])
                    rq = rowsb.tile([1, 512], bf16, tag="rowb")
                    nc.scalar.sqrt(rq[:], rc[:])
                    rb = psb.tile([128, 512], f32, tag="ps")
                    nc.tensor.matmul(rb[:], onesr_t[:], rq[:],
                                     start=True, stop=True)
                    xr = xr_t.pop(j)
                    nc.vector.tensor_mul(
                        xr8T[:, :, jsl], xr[:],
                        rb[:].rearrange("p (one q) -> p one q",
                                        one=1).broadcast_to([128, DCH, 512]))

                def TR(j):
                    for kk in range(4):
                        k = 4 * j + kk
                        tp = trps.tile([128, DCH, 256], fp8, tag="trp")
                        for dR in range(DCH):
                            outap = tp[:, dR].rearrange(
                                "p (c two) -> p c two", two=2)[:, :, 0]
                            nc.tensor.transpose(
                                outap, xr8T[:, dR, 128 * k:128 * (k + 1)],
                                id8_t[:])
                        src = tp[:].rearrange(
                            "p c (m two) -> p c m two", two=2)[:, :, :, 0]
                        dst = xr8at[:, k, :].rearrange(
                            "p (c m) -> p c m", c=DCH)
                        nc.scalar.activation(dst, src, AF.Copy)

                def B2(c):
                    # chunk c of 8: jq = c//2; half 0 = eh 0-5,
                    # half 1 = eh 6-7 + el 0-3
                    jq, hf = divmod(c, 2)
                    qsl = slice(512 * jq, 512 * (jq + 1))
                    groups = ([("h", e) for e in range(6)] if hf == 0 else
                              [("h", 6), ("h", 7)] + [("l", e)
                                                      for e in range(EL)])
                    trc = trcp.tile([128, 6, 512], bf16, tag="trc")
                    for gi, (kind, e) in enumerate(groups):
                        wt_, bt_ = (w1h_t, b1h_t) if kind == "h" else \
                                   (w1l_t, b1l_t)
                        ps = psb.tile([128, 512], f32, tag="ps")
                        for t in range(2):
                            nc.tensor.matmul(
                                ps[:],
                                wt_[:, 2 * t:2 * t + 2,
                                    128 * e:128 * (e + 1)],
                                xnT[:, 2 * t:2 * t + 2, qsl],
                                start=(t == 0), stop=(t == 1), perf_mode=PM)
                        nc.scalar.activation(trc[:, gi], ps[:], AF.Relu,
                                             bias=bt_[:, e:e + 1])
                    for gi, (kind, e) in enumerate(groups):
                        if kind == "h":
                            eng = nc.gpsimd if e % 3 != 2 else nc.vector
                            eng.tensor_mul(headT[:, e, qsl],
                                           trc[:, gi], trc[:, gi])
                        else:
                            nc.vector.tensor_mul(xlT[:, e, qsl],
                                                 trc[:, gi], trc[:, gi])

                # x blocks 0-1 queued before the bulk weight loads so compute
                # can start immediately; fuser weights load at the end of AB.
                A1(0)
                nc.sync.dma_start(w1r_t[:],
                                  w1r.rearrange("p (c m) -> p c m", c=DCH))
                A1(1)
                A1(2)
                nc.sync.dma_start(w1h_t[:],
                                  w1h.rearrange("p (c m) -> p c m", c=DCH))
                nc.sync.dma_start(w1l_t[:],
                                  w1l.rearrange("p (c m) -> p c m", c=DCH))
                for u in range(3, 12):
                    if u <= 10:
                        A2(u - 3)
                        B1a(u - 3)
                    if 6 <= u:
                        B1b(u - 6)
                        TR(u - 6)
                    if u <= 7:
                        A1(u)
                    if 4 <= u <= 11:
                        B2(u - 4)
                    if u == 7:
                        nc.sync.dma_start(
                            wfh_t[:], wfh.rearrange("p (c m) -> p c m", c=EH))
                        nc.sync.dma_start(
                            wfg_t[:], wfg.rearrange("p (c m) -> p c m", c=EL))
                B1b(6)
                TR(6)
                B1b(7)
                TR(7)

            # ============ Phase C: similarity, context, gate, fuser ========
            if phases < 2:
                return nc
            with (
                tc.tile_pool(name="ctxps", bufs=1, space="PSUM") as ctxps,
                tc.tile_pool(name="stp", bufs=4, space="PSUM") as stp,
                tc.tile_pool(name="vtp", bufs=9) as vtp,
                tc.tile_pool(name="wtp", bufs=8) as wtp,
                tc.tile_pool(name="stsb", bufs=6) as stsb,
                tc.tile_pool(name="csp", bufs=3) as csp,
                tc.tile_pool(name="gtp", bufs=3) as gtp,
                tc.tile_pool(name="outp", bufs=8) as outp,
            ):
                vt_q = {}
                ctxs = {}
                gts = {}

                def tail(qtp):
                    # sigmoid + gate for a finished qt block
                    qslp = slice(512 * qtp, 512 * (qtp + 1))
                    ctxp = ctxs[qtp]
                    cs = csp.tile([128, DCH, 512], bf16, tag="cs")
                    for dO in range(DCH):
                        nc.scalar.activation(cs[:, dO], ctxp[:, dO],
                                             AF.Sigmoid,
                                             bias=bq_t[:, dO:dO + 1],
                                             scale=1.0 / VS)
                    gt = gtp.tile([128, EL, 512], fp8, tag="gt")
                    for dO in range(DCH):
                        nc.vector.tensor_mul(gt[:, dO], xlT[:, dO, qslp],
                                             cs[:, dO])
                    gts[qtp] = gt

                def fuser_out(qtp):
                    qslp = slice(512 * qtp, 512 * (qtp + 1))
                    gt = gts.pop(qtp)
                    ctxs.pop(qtp)
                    for do in range(DCH):
                        fgd = stp.tile([128, 512], f32, tag="st")
                        for e2 in range(EH // 2):
                            nc.tensor.matmul(
                                fgd[:],
                                wfh_t[:, 2 * e2:2 * e2 + 2,
                                      128 * do:128 * (do + 1)],
                                headT[:, 2 * e2:2 * e2 + 2, qslp],
                                start=(e2 == 0), stop=False, perf_mode=PM)
                        for l2 in range(EL // 2):
                            nc.tensor.matmul(
                                fgd[:],
                                wfg_t[:, 2 * l2:2 * l2 + 2,
                                      128 * do:128 * (do + 1)],
                                gt[:, 2 * l2:2 * l2 + 2, :],
                                start=False, stop=False, perf_mode=PM)
                        nc.tensor.matmul(
                            fgd[:], idb_t[:], xall[:, do, qslp],
                            start=False, stop=True)
                        ob = outp.tile([128, 512], f32, tag="ob")
                        nc.scalar.activation(ob[:], fgd[:], AF.Copy)
                        nc.sync.dma_start(
                            outT[128 * do:128 * (do + 1), qslp], ob[:])

                for qt in range(Q512):
                    qsl = slice(512 * qt, 512 * (qt + 1))
                    ctx = ctxps.tile([128, DCH, 512], f32, tag="ctx")
                    ctxs[qt] = ctx
                    for kp in range(4):
                        vt_t = vtp.tile([128, 2, 512], fp8, tag="vt")
                        nc.scalar.dma_start(vt_t[:], vt8[:, qt, kp])
                        vt_q[kp] = vt_t

                    def emit_ctx(kp, wt, ctx=ctx):
                        for dO in range(DCH):
                            nc.tensor.matmul(
                                ctx[:, dO],
                                xr8at[:, 2 * kp:2 * kp + 2,
                                      128 * dO:128 * (dO + 1)],
                                wt[:],
                                start=(kp == 0), stop=(kp == KP - 1),
                                perf_mode=PM)

                    pendq = []
                    for kp in range(KP):
                        if kp + 4 < KP:
                            vt_t = vtp.tile([128, 2, 512], fp8, tag="vt")
                            nc.scalar.dma_start(vt_t[:], vt8[:, qt, kp + 4])
                            vt_q[kp + 4] = vt_t
                        wt = wtp.tile([128, 2, 512], fp8, tag="wt")
                        vt_t = vt_q.pop(kp)
                        for t2 in range(2):
                            k = 2 * kp + t2
                            st = stp.tile([128, 512], f32, tag="st")
                            for t in range(2):
                                nc.tensor.matmul(
                                    st[:],
                                    xr8T[:, 2 * t:2 * t + 2,
                                         128 * k:128 * (k + 1)],
                                    xr8T[:, 2 * t:2 * t + 2, qsl],
                                    start=(t == 0), stop=(t == 1),
                                    perf_mode=PM)
                            if k % 3 == 2:
                                sb2 = stsb.tile([128, 512], bf16, tag="stb")
                                nc.scalar.activation(sb2[:], st[:], AF.Copy)
                                nc.gpsimd.tensor_mul(wt[:, t2], sb2[:],
                                                     vt_t[:, t2])
                            else:
                                nc.vector.tensor_mul(wt[:, t2], st[:],
                                                     vt_t[:, t2])
                        pendq.append((kp, wt))
                        if len(pendq) > 5:
                            emit_ctx(*pendq.pop(0))
                        if qt > 0 and kp == 1:
                            tail(qt - 1)
                        if qt > 0 and kp == 4:
                            fuser_out(qt - 1)
                    while pendq:
                        emit_ctx(*pendq.pop(0))
                tail(Q512 - 1)
                fuser_out(Q512 - 1)

    return nc


_NC_CACHE = {}


def _get_nc(phases=2):
    if phases not in _NC_CACHE:
        _NC_CACHE[phases] = _build(phases)
    return _NC_CACHE[phases]


def _chunk_rows(w, nch):
    """[D=nch*128, M] -> [128, nch*M] so row p holds chunks c-major."""
    ncols = w.shape[1]
    return np.ascontiguousarray(
        w.reshape(nch, 128, ncols).transpose(1, 0, 2).reshape(128, nch * ncols))


def _prep_inputs(x, g, W1, b1, V, bias, Wf):
    x = np.asarray(x, dtype=np.float32)
    g = np.asarray(g, dtype=np.float32)
    W1 = np.asarray(W1, dtype=np.float32)
    b1 = np.asarray(b1, dtype=np.float32)
    V = np.asarray(V, dtype=np.float32)
    bias = np.asarray(bias, dtype=np.float32)
    Wf = np.asarray(Wf, dtype=np.float32)

    W1g = W1 * g[:, None]
    w1h = _chunk_rows(W1g[:, :HEAD].astype(F8), DCH)
    w1l = _chunk_rows(W1g[:, HEAD:HEAD + HALF].astype(F8), DCH)
    w1r = _chunk_rows(W1g[:, HEAD + HALF:].astype(F8), DCH)
    wfh = _chunk_rows(Wf[:HEAD].astype(F8), EH)
    wfg = _chunk_rows(Wf[HEAD:].astype(F8), EL)
    b1h = np.ascontiguousarray(b1[:HEAD].reshape(EH, 128).T)
    b1l = np.ascontiguousarray(b1[HEAD:HEAD + HALF].reshape(EL, 128).T)
    b1r = np.ascontiguousarray(b1[HEAD + HALF:].reshape(EL, 128).T)
    biasq = np.ascontiguousarray(bias.reshape(DCH, 128).T)
    id8_np = np.eye(128, dtype=F8)
    idb_np = np.eye(128, dtype=BF)
    ones_np = np.ones((128, 1), dtype=BF)
    onesr_np = np.ones((1, 128), dtype=BF)
    epsb_np = np.tile(np.array([[EPS_RMS, 1e-24]], np.float32), (128, 1))
    VT = np.ascontiguousarray(VS * V.T).astype(F8)   # VT[k, q] = 64*V[q, k]

    in_maps = []
    for c in range(NC):
        b, h = divmod(c, 2)
        q0 = h * Q
        xTb = x[b].T  # [D, N]
        if q0 == 0:
            xrot = np.ascontiguousarray(xTb).astype(BF)
            vrot = VT[:, :Q]
        else:
            # rotate tokens so own block is first; V rows rotate identically
            xrot = np.ascontiguousarray(
                np.concatenate([xTb[:, q0:], xTb[:, :q0]], axis=1)).astype(BF)
            vrot = np.concatenate([VT[q0:, q0:], VT[:q0, q0:]], axis=0)
        # [k, q] -> [p, qt, kp, t2, qq]; k = kp*256 + t2*128 + p, q = qt*512+qq
        v5 = np.ascontiguousarray(
            vrot.reshape(KP, 2, 128, Q512, 512).transpose(2, 3, 0, 1, 4))
        in_maps.append({
            "xT": xrot, "vt8": v5,
            "w1h": w1h, "w1l": w1l, "w1r": w1r,
            "wfh": wfh, "wfg": wfg,
            "b1h": b1h, "b1l": b1l, "b1r": b1r,
            "biasq": biasq, "id8": id8_np, "idb": idb_np, "onesb": ones_np,
            "onesr": onesr_np, "epsb": epsb_np,
        })
    return in_maps


def _run(in_maps, trace=False):
    nc = _get_nc()
    return run_bass_kernel_spmd(nc, in_maps, list(range(NC)), trace=trace)


def _assemble(results):
    out = np.empty((B, N, D), dtype=np.float32)
    for c in range(NC):
        b, h = divmod(c, 2)
        q0 = h * Q
        out[b, q0:q0 + Q, :] = results[c]["outT"].T
    return out


def kernel(x, g, W1, b1, V, bias, Wf):
    in_maps = _prep_inputs(x, g, W1, b1, V, bias, Wf)
    res = _run(in_maps, trace=False)
    return _assemble(res.results)


def kernel_traced(x, g, W1, b1, V, bias, Wf):
    """Same as kernel() but with NTFF tracing; returns (out, results)."""
    in_maps = _prep_inputs(x, g, W1, b1, V, bias, Wf)
    res = _run(in_maps, trace=True)
    return _assemble(res.results), res


# revision 78
# speedup vs baseline: 1.3884x; 1.0007x over previous
"""Trainium2 Bass kernel for nn_Avey_84679575208507.

Reference computation (B=4, N=4096, D=512, E=2048):
  RMSNorm -> Linear(D,E)+relu^2 -> split head/left/right ->
  cosine-sim attention vs learned positional V -> sigmoid gate ->
  Linear(1536,D) + residual.

Sharding: data-parallel over batch x sequence-half; each of 8 cores owns
(batch b = core//2, rows q0 = (core%2)*2048 .. +2048). Tensors are kept in
transposed layout [feature, token] on chip; x and V are pre-transposed (and
token-rotated so the own block is always columns [0, Q)) on the host, g is
folded into W1.

v4: x ships as bf16 and loads in 8 contiguous strips into a resident tile;
every large matmul (enricher linears, similarity S = xr_n^T xr_n, context
ctx = xr_n_at^T (V*S), and the fuser) runs in fp8-e4m3 with DoubleRow perf
mode (2 k-tiles contracted per instruction); V is pre-scaled by 64 into fp8
(undone in the sigmoid's scale). Phases A/B1 (RMSNorm + right path) are
software-pipelined 3-6 blocks deep per 512-token block with on-chip PE
rank-1 broadcasts for the per-token scales; the head/left linears interleave
as PE fill work. Phase C trails the ctx accumulation one key-pair behind the
S matmuls, splits the V*S elementwise product 2:1 across DVE and
(ACT-copy + Pool), and pipelines each query block's sigmoid/gate/fuser tail
inside the next block's key loop; the fuser + residual accumulate in a
single PSUM group per output tile. End-to-end rel err ~1.1e-2 (CPU-model
verified) against the fp32 reference, within the 2e-2 gate.
"""

import sys

sys.path.insert(0, "/opt/trn_rl_repo")

import numpy as np
import ml_dtypes

import concourse.bass as bass
import concourse.tile as tile
import concourse.mybir as mybir
from concourse.bass_utils import run_bass_kernel_spmd

f32 = mybir.dt.float32
bf16 = mybir.dt.bfloat16
fp8 = mybir.dt.float8e4
AF = mybir.ActivationFunctionType
PM = mybir.MatmulPerfMode.DoubleRow
BF = ml_dtypes.bfloat16
F8 = ml_dtypes.float8_e4m3

B, N, D = 4, 4096, 512
E = 4 * D          # 2048
TAIL = E // 2      # 1024
HALF = TAIL // 2   # 512
HEAD = E - TAIL    # 1024
EPS_RMS = 1e-6
Q = N // 2         # 2048 own rows per core
NC = 8
DCH = D // 128     # 4 partition chunks of d
N512 = N // 512    # 8 token blocks
Q512 = Q // 512    # 4 own token blocks
KCH = N // 128     # 32 key chunks
KP = KCH // 2      # 16 key-chunk pairs
EH = HEAD // 128   # 8 head e' chunks
EL = HALF // 128   # 4 left e' chunks
VS = 64.0          # host pre-scale on V (fp8 range); undone in sigmoid scale


def _split_multi_waits(nc):
    """Walrus in this container accepts only one sync-wait per instruction;
    hoist extra waits onto single-wait NoOps just before, same engine."""
    n = 0
    for fn in nc.m.functions:
        for blk in fn.blocks:
            out = []
            for inst in blk.instructions:
                si = inst.sync_info
                if si is not None and si.on_wait and len(si.on_wait) > 1:
                    waits = list(si.on_wait)[::-1]
                    for i, w in enumerate(waits[:-1]):
                        out.append(mybir.InstNoOp(
                            name=f"{inst.name}_wsplit{i}",
                            engine=inst.engine,
                            bass_nofuse=True,
                            sync_info=mybir.SyncInfo(on_wait=[w], on_update=[]),
                        ))
                    inst.sync_info = mybir.SyncInfo(
                        on_wait=[waits[-1]], on_update=list(si.on_update or []))
                    n += 1
                out.append(inst)
            blk.instructions = out
    return n


def _build(phases=2):
    nc = _build_inner(phases)
    _split_multi_waits(nc)
    return nc


def _build_inner(phases=2):
    nc = bass.Bass("TRN2", target_bir_lowering=False, debug=False, num_devices=NC)

    xT = nc.dram_tensor("xT", [D, N], bf16, kind="ExternalInput").ap()
    # [p, qt, kp, t2, qq] pre-arranged so each C-phase load is contiguous
    vt8 = nc.dram_tensor("vt8", [128, Q512, KP, 2, 512], fp8,
                         kind="ExternalInput").ap()
    # weights pre-arranged host-side to [128, chunks*cols] contiguous rows
    w1h = nc.dram_tensor("w1h", [128, DCH * HEAD], fp8, kind="ExternalInput").ap()
    w1l = nc.dram_tensor("w1l", [128, DCH * HALF], fp8, kind="ExternalInput").ap()
    w1r = nc.dram_tensor("w1r", [128, DCH * HALF], fp8, kind="ExternalInput").ap()
    wfh = nc.dram_tensor("wfh", [128, EH * D], fp8, kind="ExternalInput").ap()
    wfg = nc.dram_tensor("wfg", [128, EL * D], fp8, kind="ExternalInput").ap()
    b1h = nc.dram_tensor("b1h", [128, EH], f32, kind="ExternalInput").ap()
    b1l = nc.dram_tensor("b1l", [128, EL], f32, kind="ExternalInput").ap()
    b1r = nc.dram_tensor("b1r", [128, EL], f32, kind="ExternalInput").ap()
    biasq = nc.dram_tensor("biasq", [128, DCH], f32, kind="ExternalInput").ap()
    id8 = nc.dram_tensor("id8", [128, 128], fp8, kind="ExternalInput").ap()
    idb = nc.dram_tensor("idb", [128, 128], bf16, kind="ExternalInput").ap()
    onesb = nc.dram_tensor("onesb", [128, 1], bf16, kind="ExternalInput").ap()
    onesr = nc.dram_tensor("onesr", [1, 128], bf16, kind="ExternalInput").ap()
    epsb = nc.dram_tensor("epsb", [128, 2], f32, kind="ExternalInput").ap()
    outT = nc.dram_tensor("outT", [D, Q], f32, kind="ExternalOutput").ap()

    with tile.TileContext(nc) as tc:
        with (
            tc.tile_pool(name="consts", bufs=1) as consts,
            tc.tile_pool(name="wfp", bufs=1) as wfp,
            tc.tile_pool(name="xr8Tp", bufs=1) as xr8Tp,
            tc.tile_pool(name="xr8atp", bufs=1) as xr8atp,
            tc.tile_pool(name="xlTp", bufs=1) as xlTp,
            tc.tile_pool(name="headTp", bufs=1) as headTp,
            tc.tile_pool(name="xofp", bufs=1) as xofp,
        ):
            id8_t = consts.tile([128, 128], fp8)
            nc.sync.dma_start(id8_t[:], id8[:])
            idb_t = consts.tile([128, 128], bf16)
            nc.sync.dma_start(idb_t[:], idb[:])
            ones_t = consts.tile([128, 1], bf16)
            nc.sync.dma_start(ones_t[:], onesb[:])
            onesr_t = consts.tile([1, 128], bf16)
            nc.sync.dma_start(onesr_t[:], onesr[:])
            b1h_t = consts.tile([128, EH], f32)
            nc.sync.dma_start(b1h_t[:], b1h[:])
            b1l_t = consts.tile([128, EL], f32)
            nc.sync.dma_start(b1l_t[:], b1l[:])
            b1r_t = consts.tile([128, EL], f32)
            nc.sync.dma_start(b1r_t[:], b1r[:])
            bq_t = consts.tile([128, DCH], f32)
            nc.sync.dma_start(bq_t[:], biasq[:])
            eps_t = consts.tile([128, 2], f32)
            nc.sync.dma_start(eps_t[:], epsb[:])
            wfh_t = wfp.tile([128, EH, D], fp8)
            wfg_t = wfp.tile([128, EL, D], fp8)

            xr8T = xr8Tp.tile([128, DCH, N], fp8)
            xr8at = xr8atp.tile([128, KCH, HALF], fp8)
            xlT = xlTp.tile([128, EL, Q], bf16)
            headT = headTp.tile([128, EH, Q], fp8)
            xall = xofp.tile([128, DCH, N], bf16)
            for i_ in range(DCH):
                for h_ in range(2):
                    nc.scalar.dma_start(
                        xall[:, i_, 2048 * h_:2048 * (h_ + 1)],
                        xT[128 * i_:128 * (i_ + 1),
                           2048 * h_:2048 * (h_ + 1)])

            # ============ Phase AB: pipelined RMSNorm + right path +
            # ============ head/left, per 512-token block j ============
            with (
                tc.tile_pool(name="w1p", bufs=1) as w1p,
                tc.tile_pool(name="xnp", bufs=1) as xnp,
                tc.tile_pool(name="sqp", bufs=3) as sqp,
                tc.tile_pool(name="xrp", bufs=5) as xrp,
                tc.tile_pool(name="trp", bufs=3) as trp,
                tc.tile_pool(name="trcp", bufs=2) as trcp,
                tc.tile_pool(name="rowsb", bufs=2) as rowsb,
                tc.tile_pool(name="rowps", bufs=2, space="PSUM") as rowps,
                tc.tile_pool(name="psb", bufs=4, space="PSUM") as psb,
                tc.tile_pool(name="trps", bufs=2, space="PSUM") as trps,
            ):
                w1h_t = w1p.tile([128, DCH, HEAD], fp8)
                w1l_t = w1p.tile([128, DCH, HALF], fp8)
                w1r_t = w1p.tile([128, DCH, HALF], fp8)
                xnT = xnp.tile([128, DCH, N], fp8)

                xc_t = {}
                xr_t = {}
                ms_t = {}
                rsl_t = {}

                def A1(j):
                    jsl = slice(512 * j, 512 * (j + 1))
                    ss = rowps.tile([1, 512], f32, tag="row")
                    xc = xall[:, :, jsl]
                    xc_t[j] = xc
                    xsq = sqp.tile([128, DCH, 512], bf16, tag="xsq")
                    for i in range(DCH):
                        nc.vector.tensor_mul(xsq[:, i], xc[:, i], xc[:, i])
                        nc.tensor.matmul(ss[0:1, :], ones_t[:], xsq[:, i],
                                         start=(i == 0), stop=(i == DCH - 1))
                    ms = rowsb.tile([1, 512], f32, tag="rowf")
                    nc.scalar.activation(ms[:], ss[0:1, :], AF.Identity,
                                         bias=eps_t[0:1, 0:1], scale=1.0 / D)
                    ms_t[j] = ms

                def A2(j):
                    jsl = slice(512 * j, 512 * (j + 1))
                    rr = rowsb.tile([1, 512], f32, tag="rowf2")
                    nc.vector.reciprocal(rr[:], ms_t.pop(j)[:])
                    sr = rowsb.tile([1, 512], bf16, tag="rowb")
                    nc.scalar.sqrt(sr[:], rr[:])
                    # broadcast row across partitions on PE: ones[1,128] x row
                    sb = psb.tile([128, 512], f32, tag="ps")
                    nc.tensor.matmul(sb[:], onesr_t[:], sr[:],
                                     start=True, stop=True)
                    xc = xc_t.pop(j)
                    nc.vector.tensor_mul(
                        xnT[:, :, jsl], xc,
                        sb[:].rearrange("p (one q) -> p one q",
                                        one=1).broadcast_to([128, DCH, 512]))

                def B1a(j):
                    jsl = slice(512 * j, 512 * (j + 1))
                    rs = rowps.tile([1, 512], f32, tag="row")
                    tr = trp.tile([128, DCH, 512], bf16, tag="tr")
                    for dR in range(DCH):
                        ps = psb.tile([128, 512], f32, tag="ps")
                        for t in range(2):
                            nc.tensor.matmul(
                                ps[:],
                                w1r_t[:, 2 * t:2 * t + 2,
                                      128 * dR:128 * (dR + 1)],
                                xnT[:, 2 * t:2 * t + 2, jsl],
                                start=(t == 0), stop=(t == 1), perf_mode=PM)
                        nc.scalar.activation(tr[:, dR], ps[:], AF.Relu,
                                             bias=b1r_t[:, dR:dR + 1])
                    xr = xrp.tile([128, DCH, 512], bf16, tag="xr")
                    xr_t[j] = xr
                    xq = sqp.tile([128, DCH, 512], bf16, tag="xsq")
                    for dR in range(DCH):
                        nc.vector.tensor_mul(xr[:, dR], tr[:, dR], tr[:, dR])
                        nc.gpsimd.tensor_mul(xq[:, dR], xr[:, dR], xr[:, dR])
                    for dR in range(DCH):
                        nc.tensor.matmul(rs[0:1, :], ones_t[:], xq[:, dR],
                                         start=(dR == 0), stop=(dR == DCH - 1))
                    rsl = rowsb.tile([1, 512], f32, tag="rowf")
                    # 1/max(sqrt(S),1e-12) == 1/sqrt(S+1e-24) in fp32
                    nc.scalar.activation(rsl[:], rs[0:1, :], AF.Identity,
                                         bias=eps_t[0:1, 1:2])
                    rsl_t[j] = rsl

                def B1b(j):
                    jsl = slice(512 * j, 512 * (j + 1))
                    rc = rowsb.tile([1, 512], f32, tag="rowf2")
                    nc.vector.reciprocal(rc[:], rsl_t.pop(j)[:])
                    rq = rowsb.tile([1, 512], bf16, tag="rowb")
                    nc.scalar.sqrt(rq[:], rc[:])
                    rb = psb.tile([128, 512], f32, tag="ps")
                    nc.tensor.matmul(rb[:], onesr_t[:], rq[:],
                                     start=True, stop=True)
                    xr = xr_t.pop(j)
                    nc.vector.tensor_mul(
                        xr8T[:, :, jsl], xr[:],
                        rb[:].rearrange("p (one q) -> p one q",
                                        one=1).broadcast_to([128, DCH, 512]))

                def TR(j):
                    for kk in range(4):
                        k = 4 * j + kk
                        tp = trps.tile([128, DCH, 256], fp8, tag="trp")
                        for dR in range(DCH):
                            outap = tp[:, dR].rearrange(
                                "p (c two) -> p c two", two=2)[:, :, 0]
                            nc.tensor.transpose(
                                outap, xr8T[:, dR, 128 * k:128 * (k + 1)],
                                id8_t[:])
                        src = tp[:].rearrange(
                            "p c (m two) -> p c m two", two=2)[:, :, :, 0]
                        dst = xr8at[:, k, :].rearrange(
                            "p (c m) -> p c m", c=DCH)
                        nc.scalar.activation(dst, src, AF.Copy)

                def B2(c):
                    # chunk c of 8: jq = c//2; half 0 = eh 0-5,
                    # half 1 = eh 6-7 + el 0-3
                    jq, hf = divmod(c, 2)
                    qsl = slice(512 * jq, 512 * (jq + 1))
                    groups = ([("h", e) for e in range(6)] if hf == 0 else
                              [("h", 6), ("h", 7)] + [("l", e)
                                                      for e in range(EL)])
                    trc = trcp.tile([128, 6, 512], bf16, tag="trc")
                    for gi, (kind, e) in enumerate(groups):
                        wt_, bt_ = (w1h_t, b1h_t) if kind == "h" else \
                                   (w1l_t, b1l_t)
                        ps = psb.tile([128, 512], f32, tag="ps")
                        for t in range(2):
                            nc.tensor.matmul(
                                ps[:],
                                wt_[:, 2 * t:2 * t + 2,
                                    128 * e:128 * (e + 1)],
                                xnT[:, 2 * t:2 * t + 2, qsl],
                                start=(t == 0), stop=(t == 1), perf_mode=PM)
                        nc.scalar.activation(trc[:, gi], ps[:], AF.Relu,
                                             bias=bt_[:, e:e + 1])
                    for gi, (kind, e) in enumerate(groups):
                        if kind == "h":
                            eng = nc.gpsimd if e % 3 != 2 else nc.vector
                            eng.tensor_mul(headT[:, e, qsl],
                                           trc[:, gi], trc[:, gi])
                        else:
                            nc.vector.tensor_mul(xlT[:, e, qsl],
                                                 trc[:, gi], trc[:, gi])

                # x blocks 0-1 queued before the bulk weight loads so compute
                # can start immediately; fuser weights load at the end of AB.
                A1(0)
                nc.sync.dma_start(w1r_t[:],
                                  w1r.rearrange("p (c m) -> p c m", c=DCH))
                A1(1)
                A1(2)
                nc.sync.dma_start(w1h_t[:],
                                  w1h.rearrange("p (c m) -> p c m", c=DCH))
                nc.sync.dma_start(w1l_t[:],
                                  w1l.rearrange("p (c m) -> p c m", c=DCH))
                for u in range(3, 12):
                    if u <= 10:
                        A2(u - 3)
                        B1a(u - 3)
                    if 6 <= u:
                        B1b(u - 6)
                        TR(u - 6)
                    if u <= 7:
                        A1(u)
                    if 4 <= u <= 11:
                        B2(u - 4)
                    if u == 7:
                        nc.sync.dma_start(
                            wfh_t[:], wfh.rearrange("p (c m) -> p c m", c=EH))
                        nc.sync.dma_start(
                            wfg_t[:], wfg.rearrange("p (c m) -> p c m", c=EL))
                B1b(6)
                TR(6)
                B1b(7)
                TR(7)

            # ============ Phase C: similarity, context, gate, fuser ========
            if phases < 2:
                return nc
            with (
                tc.tile_pool(name="ctxps", bufs=1, space="PSUM") as ctxps,
                tc.tile_pool(name="stp", bufs=4, space="PSUM") as stp,
                tc.tile_pool(name="vtp", bufs=9) as vtp,
                tc.tile_pool(name="wtp", bufs=8) as wtp,
                tc.tile_pool(name="stsb", bufs=6) as stsb,
                tc.tile_pool(name="csp", bufs=3) as csp,
                tc.tile_pool(name="gtp", bufs=3) as gtp,
                tc.tile_pool(name="outp", bufs=8) as outp,
            ):
                vt_q = {}
                ctxs = {}
                gts = {}

                def tail(qtp):
                    # sigmoid + gate for a finished qt block
                    qslp = slice(512 * qtp, 512 * (qtp + 1))
                    ctxp = ctxs[qtp]
                    cs = csp.tile([128, DCH, 512], bf16, tag="cs")
                    for dO in range(DCH):
                        nc.scalar.activation(cs[:, dO], ctxp[:, dO],
                                             AF.Sigmoid,
                                             bias=bq_t[:, dO:dO + 1],
                                             scale=1.0 / VS)
                    gt = gtp.tile([128, EL, 512], fp8, tag="gt")
                    for dO in range(DCH):
                        nc.vector.tensor_mul(gt[:, dO], xlT[:, dO, qslp],
                                             cs[:, dO])
                    gts[qtp] = gt

                def fuser_out(qtp):
                    qslp = slice(512 * qtp, 512 * (qtp + 1))
                    gt = gts.pop(qtp)
                    ctxs.pop(qtp)
                    for do in range(DCH):
                        fgd = stp.tile([128, 512], f32, tag="st")
                        for e2 in range(EH // 2):
                            nc.tensor.matmul(
                                fgd[:],
                                wfh_t[:, 2 * e2:2 * e2 + 2,
                                      128 * do:128 * (do + 1)],
                                headT[:, 2 * e2:2 * e2 + 2, qslp],
                                start=(e2 == 0), stop=False, perf_mode=PM)
                        for l2 in range(EL // 2):
                            nc.tensor.matmul(
                                fgd[:],
                                wfg_t[:, 2 * l2:2 * l2 + 2,
                                      128 * do:128 * (do + 1)],
                                gt[:, 2 * l2:2 * l2 + 2, :],
                                start=False, stop=False, perf_mode=PM)
                        nc.tensor.matmul(
                            fgd[:], idb_t[:], xall[:, do, qslp],
                            start=False, stop=True)
                        ob = outp.tile([128, 512], f32, tag="ob")
                        nc.scalar.activation(ob[:], fgd[:], AF.Copy)
                        nc.sync.dma_start(
                            outT[128 * do:128 * (do + 1), qslp], ob[:])

                for qt in range(Q512):
                    qsl = slice(512 * qt, 512 * (qt + 1))
                    ctx = ctxps.tile([128, DCH, 512], f32, tag="ctx")
                    ctxs[qt] = ctx
                    for kp in range(4):
                        vt_t = vtp.tile([128, 2, 512], fp8, tag="vt")
                        nc.scalar.dma_start(vt_t[:], vt8[:, qt, kp])
                        vt_q[kp] = vt_t

                    def emit_ctx(kp, wt, ctx=ctx):
                        for dO in range(DCH):
                            nc.tensor.matmul(
                                ctx[:, dO],
                                xr8at[:, 2 * kp:2 * kp + 2,
                                      128 * dO:128 * (dO + 1)],
                                wt[:],
                                start=(kp == 0), stop=(kp == KP - 1),
                                perf_mode=PM)

                    pendq = []
                    for kp in range(KP):
                        if kp + 4 < KP:
                            vt_t = vtp.tile([128, 2, 512], fp8, tag="vt")
                            nc.scalar.dma_start(vt_t[:], vt8[:, qt, kp + 4])
                            vt_q[kp + 4] = vt_t
                        wt = wtp.tile([128, 2, 512], fp8, tag="wt")
                        vt_t = vt_q.pop(kp)
                        for t2 in range(2):
                            k = 2 * kp + t2
                            st = stp.tile([128, 512], f32, tag="st")
                            for t in range(2):
                                nc.tensor.matmul(
                                    st[:],
                                    xr8T[:, 2 * t:2 * t + 2,
                                         128 * k:128 * (k + 1)],
                                    xr8T[:, 2 * t:2 * t + 2, qsl],
                                    start=(t == 0), stop=(t == 1),
                                    perf_mode=PM)
                            if k % 3 == 2:
                                sb2 = stsb.tile([128, 512], bf16, tag="stb")
                                nc.scalar.activation(sb2[:], st[:], AF.Copy)
                                nc.gpsimd.tensor_mul(wt[:, t2], sb2[:],
                                                     vt_t[:, t2])
                            else:
                                nc.vector.tensor_mul(wt[:, t2], st[:],
                                                     vt_t[:, t2])
                        pendq.append((kp, wt))
                        if len(pendq) > 5:
                            emit_ctx(*pendq.pop(0))
                        if qt > 0 and kp == 1:
                            tail(qt - 1)
                        if qt > 0 and kp == 4:
                            fuser_out(qt - 1)
                    while pendq:
                        emit_ctx(*pendq.pop(0))
                tail(Q512 - 1)
                fuser_out(Q512 - 1)

    return nc


_NC_CACHE = {}


def _get_nc(phases=2):
    if phases not in _NC_CACHE:
        _NC_CACHE[phases] = _build(phases)
    return _NC_CACHE[phases]


def _chunk_rows(w, nch):
    """[D=nch*128, M] -> [128, nch*M] so row p holds chunks c-major."""
    ncols = w.shape[1]
    return np.ascontiguousarray(
        w.reshape(nch, 128, ncols).transpose(1, 0, 2).reshape(128, nch * ncols))


def _prep_inputs(x, g, W1, b1, V, bias, Wf):
    x = np.asarray(x, dtype=np.float32)
    g = np.asarray(g, dtype=np.float32)
    W1 = np.asarray(W1, dtype=np.float32)
    b1 = np.asarray(b1, dtype=np.float32)
    V = np.asarray(V, dtype=np.float32)
    bias = np.asarray(bias, dtype=np.float32)
    Wf = np.asarray(Wf, dtype=np.float32)

    W1g = W1 * g[:, None]
    w1h = _chunk_rows(W1g[:, :HEAD].astype(F8), DCH)
    w1l = _chunk_rows(W1g[:, HEAD:HEAD + HALF].astype(F8), DCH)
    w1r = _chunk_rows(W1g[:, HEAD + HALF:].astype(F8), DCH)
    wfh = _chunk_rows(Wf[:HEAD].astype(F8), EH)
    wfg = _chunk_rows(Wf[HEAD:].astype(F8), EL)
    b1h = np.ascontiguousarray(b1[:HEAD].reshape(EH, 128).T)
    b1l = np.ascontiguousarray(b1[HEAD:HEAD + HALF].reshape(EL, 128).T)
    b1r = np.ascontiguousarray(b1[HEAD + HALF:].reshape(EL, 128).T)
    biasq = np.ascontiguousarray(bias.reshape(DCH, 128).T)
    id8_np = np.eye(128, dtype=F8)
    idb_np = np.eye(128, dtype=BF)
    ones_np = np.ones((128, 1), dtype=BF)
    onesr_np = np.ones((1, 128), dtype=BF)
    epsb_np = np.tile(np.array([[EPS_RMS, 1e-24]], np.float32), (128, 1))
    VT = np.ascontiguousarray(VS * V.T).astype(F8)   # VT[k, q] = 64*V[q, k]

    in_maps = []
    for c in range(NC):
        b, h = divmod(c, 2)
        q0 = h * Q
        xTb = x[b].T  # [D, N]
        if q0 == 0:
            xrot = np.ascontiguousarray(xTb).astype(BF)
            vrot = VT[:, :Q]
        else:
            # rotate tokens so own block is first; V rows rotate identically
            xrot = np.ascontiguousarray(
                np.concatenate([xTb[:, q0:], xTb[:, :q0]], axis=1)).astype(BF)
            vrot = np.concatenate([VT[q0:, q0:], VT[:q0, q0:]], axis=0)
        # [k, q] -> [p, qt, kp, t2, qq]; k = kp*256 + t2*128 + p, q = qt*512+qq
        v5 = np.ascontiguousarray(
            vrot.reshape(KP, 2, 128, Q512, 512).transpose(2, 3, 0, 1, 4))
        in_maps.append({
            "xT": xrot, "vt8": v5,
            "w1h": w1h, "w1l": w1l, "w1r": w1r,
            "wfh": wfh, "wfg": wfg,
            "b1h": b1h, "b1l": b1l, "b1r": b1r,
            "biasq": biasq, "id8": id8_np, "idb": idb_np, "onesb": ones_np,
            "onesr": onesr_np, "epsb": epsb_np,
        })
    return in_maps


def _run(in_maps, trace=False):
    nc = _get_nc()
    return run_bass_kernel_spmd(nc, in_maps, list(range(NC)), trace=trace)


def _assemble(results):
    out = np.empty((B, N, D), dtype=np.float32)
    for c in range(NC):
        b, h = divmod(c, 2)
        q0 = h * Q
        out[b, q0:q0 + Q, :] = results[c]["outT"].T
    return out


def kernel(x, g, W1, b1, V, bias, Wf):
    in_maps = _prep_inputs(x, g, W1, b1, V, bias, Wf)
    res = _run(in_maps, trace=False)
    return _assemble(res.results)


def kernel_traced(x, g, W1, b1, V, bias, Wf):
    """Same as kernel() but with NTFF tracing; returns (out, results)."""
    in_maps = _prep_inputs(x, g, W1, b1, V, bias, Wf)
    res = _run(in_maps, trace=True)
    return _assemble(res.results), res
